# revision 58
# baseline (speedup 1.0000x reference)
"""BinaryLSTM (binary tree-LSTM cell) Trainium2 kernel.

Full-input contract: kernel(**inputs) takes the complete unsharded tensors and
returns (h, c), each [8192, 1024] float32, matching the reference.

Strategy
--------
Data-parallel over the batch dim: core r handles rows r*1024:(r+1)*1024.
Gate pre-activations are computed as z[h, b] = sum_k V[k, h] * XT[k, b] with
X = [p | hl | hr] ([B, 3072]), fp16 matmuls, PSUM [128, 512] banks, per-gate
per-m bias fused into the ACT sigmoid/tanh (baseline design, 409.6us).

This version cuts PE work ~8% with a Strassen-Winograd decomposition of the
three K=3072 gates (i / u / o).  Each gate GEMM [H=1024, K=3072, B=1024] is
split 2x2x2; Winograd's 7-multiply form needs 4 B-side combos
(T1 = B12-B11, T2 = B22-T1, T3 = B22-B12, T4 = T2-B21), built ONCE on DVE
from XT and reused by all 3 gates x 4 row-groups, and 7 PSUM-in-place U-folds
per group on DVE (idle ~75%).  A-side combos are free (host-precomputed into
the weight stream; 7/4 more weight bytes, amply covered by DMA headroom).
Products are ordered M1,M2,M6,M7,M5,M3,M4 so each fold fires right after its
last operand's product completes; C11 activates ~a third into the group, the
rest near the end, overlapping the next group's matmuls.

DVE reads at most ONE PSUM operand per instruction (hw port limit), so M1 is
ACT-staged to SBUF, U2/U3/U4 are computed PSUM+SBUF->SBUF f16 (freeing their
banks immediately), and the four C-blocks land PSUM+SBUF->PSUM for the ACT.

Outer loop runs over ph = 0..3 since one Strassen group yields gate rows for
both m=ph and m=ph+4.  The kernel front is DMA-bound (~250-280 GB/s/core
effective while all 8 cores fetch identical weights): pf gemms - the only
work needing just the first 2MB of XT - are hoisted one iteration ahead and
parked in SBUF f16, filling the PE while the 6.3MB XT stream lands.  fc is
built in place over the cl half of the clcr tile to stay inside SBUF.  The
last iteration computes o straight (n-outer) so only ACT(o) -> h mul ->
small h DMA trail the final matmul.

Every DMA trigger is emitted after the program point whose completion frees
its ring slot: the Sync engine processes triggers in order, so a trigger
waiting on a future iteration's consumer deadlocks the device (observed).
cl/cr fuse into one input DMA per m and h/c into one output DMA per m.
"""

import os
import sys

for _p in ("/opt/trn_rl_repo", "/root/.axon_site/_ro/trn_rl_repo"):
    if os.path.isdir(_p) and _p not in sys.path:
        sys.path.append(_p)

import numpy as np

import concourse.bass as bass
import concourse.tile as tile
import concourse.mybir as mybir
from concourse import bacc
from concourse import bass_utils

B, D, H = 8192, 1024, 1024
NCORES = 8
BL = B // NCORES            # 1024 batch rows per core
K3 = 3 * D                  # 3072 contraction (p | hl | hr)
KT = K3 // 128              # 24 k-tiles
MT = H // 128               # 8 h-tiles (PSUM partition dim)
NFREE = 512                 # moving free dim per matmul (one PSUM bank, fp32)
NT = BL // NFREE            # 2 b-tiles

F32 = mybir.dt.float32
F16 = mybir.dt.float16

_CACHE = {}

# Results of the most recent hardware run (for test harness introspection).
LAST_RESULTS = None

# bias column index per activated gate (bt layout: col = gi*MT + m)
BIAS_IDX = {"i": 0, "fl": 1, "fr": 2, "u": 3, "o": 4}

# Winograd product order (consumption order): M1, M2, M6, M7, M5, M3, M4.
# A-side matrix per jj: A11, A12, S2, S3, S1, S4, A22 (packed on host).
# Moving operand per jj: B11, B21, T2, T3, T1, B22, T4.
NJJ = 7
KJJ = 12                    # k-tiles per product (K/2 = 1536)


def _build_program():
    nc = bacc.Bacc("TRN2", target_bir_lowering=False, debug=False,
                   num_devices=NCORES)

    xt_d = nc.dram_tensor("xt", [K3, BL], F16, kind="ExternalInput").ap()
    wstr_d = nc.dram_tensor("w_str", [12, 128, NJJ * KJJ * 128], F16,
                            kind="ExternalInput").ap()
    flfr_d = nc.dram_tensor("w_flfr", [MT, 128, 32 * 128], F16,
                            kind="ExternalInput").ap()
    wpf_d = nc.dram_tensor("w_pf", [4, 128, 16 * 128], F16,
                           kind="ExternalInput").ap()
    wo3_d = nc.dram_tensor("w_o3", [128, 48 * 128], F16,
                           kind="ExternalInput").ap()
    clcr_d = nc.dram_tensor("clcr", [H, 2 * BL], F16,
                            kind="ExternalInput").ap()
    bt_d = nc.dram_tensor("bt", [128, 5 * MT], F32, kind="ExternalInput").ap()
    hct_d = nc.dram_tensor("hct", [H, 2 * BL], F16,
                           kind="ExternalOutput").ap()

    SIG = mybir.ActivationFunctionType.Sigmoid
    TANH = mybir.ActivationFunctionType.Tanh

    with tile.TileContext(nc) as tc:
        with tc.tile_pool(name="const", bufs=1) as const_pool, \
             tc.tile_pool(name="xtp", bufs=1) as xt_pool, \
             tc.tile_pool(name="tcb", bufs=1) as t_pool, \
             tc.tile_pool(name="wp", bufs=2) as w_pool, \
             tc.tile_pool(name="gp", bufs=2) as g_pool, \
             tc.tile_pool(name="ep", bufs=2) as e_pool, \
             tc.tile_pool(name="pp", bufs=8, space="PSUM") as p_pool:

            xt_r = xt_d.rearrange("(k p) b -> p k b", p=128)
            xt_t = xt_pool.tile([128, KT, BL], F16, name="xt_all", tag="x",
                                bufs=1)

            # PE warmup: burn the pstate ramp on throwaway matmuls during
            # the DMA lead-in (PE is idle until first weights/xt land).
            # Short warmup only: the hoisted pf gemms below provide real
            # work to burn the pstate ramp against the front DMA wall.
            warm_w = const_pool.tile([128, 128], F16, name="warm_w")
            nc.vector.memset(warm_w[:], 0)
            warm_ps = p_pool.tile([128, 128], F32, name="warm_ps",
                                  tag="ps")
            # Bridge the PE all the way to first-data (~14.8us): the 5.9us
            # idle is DMA-bound either way, but a continuously-busy PE keeps
            # full pstate so the real pf matmuls start at 2.4GHz instead of
            # re-ramping (~1.2us tax).  Sized to slightly undershoot.
            for _ in range(112):
                nc.tensor.matmul(warm_ps[:], warm_w[:], warm_w[:],
                                 start=True, stop=True)

            def load_x(k0, k1):
                nc.sync.dma_start(xt_t[:, k0:k1, :], xt_r[:, k0:k1, :])

            # --- initial DMA emission, sequenced for the critical path ---
            wpf_tiles = {}

            def load_wpf_pair(pi):
                """pf weights for the m-pair (pi, pi+4) in one DMA."""
                t = w_pool.tile([128, 16, 128], F16, name=f"wpf_{pi}",
                                tag="wpf", bufs=2)
                nc.sync.dma_start(t[:], wpf_d[pi].rearrange(
                    "p (k c) -> p k c", k=16))
                wpf_tiles[pi] = t

            clcr_tiles = {}

            def load_clcr(m):
                t = e_pool.tile([128, 2 * BL], F16, name=f"clcr_{m}",
                                tag="clcr", bufs=2)
                nc.sync.dma_start(t[:], clcr_d[m * 128:(m + 1) * 128, :])
                clcr_tiles[m] = t

            flfr_tiles = {}

            def load_flfr(m):
                """Fused [fl|fr] weights: the 2-slot ring only ever waits on
                same-iteration gemms (a finer split deadlocked the Sync
                queue against a next-iteration consumer)."""
                t = w_pool.tile([128, 32, 128], F16, name=f"flfr_{m}",
                                tag="wfl", bufs=2)
                nc.sync.dma_start(t[:], flfr_d[m].rearrange(
                    "p (k c) -> p k c", k=32))
                flfr_tiles[m] = t

            ws_tiles = {}

            def load_ws(g, ph, split=False):
                t = w_pool.tile([128, NJJ * KJJ, 128], F16,
                                name=f"ws_{g}_{ph}", tag="ws", bufs=2)
                r = wstr_d[g * 4 + ph].rearrange("p (k c) -> p k c",
                                                 k=NJJ * KJJ)
                if split:
                    # first products usable before the whole block lands
                    nc.sync.dma_start(t[:, 0:42, :], r[:, 0:42, :])
                    nc.sync.dma_start(t[:, 42:84, :], r[:, 42:84, :])
                else:
                    nc.sync.dma_start(t[:], r)
                ws_tiles[(g, ph)] = t

            # xt is the bandwidth-bound lead-in (6.3MB): trigger all its
            # slices early; ~1us of Sync time per trigger delays the stream.
            load_wpf_pair(0)
            load_x(0, 2)
            load_x(2, 5)
            load_x(5, 8)
            load_wpf_pair(1)
            load_flfr(0)
            load_x(8, 12)
            load_x(12, 18)
            load_x(18, 24)
            # i-group ph0, split: products jj0..3 start on the first half
            # (~3.5us earlier than the full 2.75MB block would land)
            load_ws(0, 0, split=True)
            bt_t = const_pool.tile([128, 5 * MT], F32, name="bt_t")
            nc.sync.dma_start(bt_t[:], bt_d)
            load_flfr(4)
            load_ws(1, 0)       # u-group, ph 0
            # clcr feeds only fc (DVE, consumed ~88us in): lowest priority
            load_clcr(0)
            load_clcr(4)

            # --- B-side Winograd combos (fp16, SBUF), built once on DVE ---
            # T1/T2 queue behind the xt DMAs; T3/T4 are emitted later (after
            # iter0 m=0 folds) so the pf/fl folds are not stuck behind them.
            t1_t = t_pool.tile([128, KJJ, NFREE], F16, name="t1", tag="t1")
            t2_t = t_pool.tile([128, KJJ, NFREE], F16, name="t2", tag="t2")
            t3_t = t_pool.tile([128, KJJ, NFREE], F16, name="t3", tag="t3")
            t4_t = t_pool.tile([128, KJJ, NFREE], F16, name="t4", tag="t4")
            b11 = xt_t[:, 0:12, 0:512]
            b12 = xt_t[:, 0:12, 512:1024]
            b21 = xt_t[:, 12:24, 0:512]
            b22 = xt_t[:, 12:24, 512:1024]
            nc.vector.tensor_sub(t1_t[:], b12, b11)
            nc.vector.tensor_sub(t2_t[:], b22, t1_t[:])

            def moving(jj, k):
                """Moving operand per product: B11, B21, T2, T3, T1, B22, T4."""
                if jj == 0:
                    return xt_t[:, k, 0:512]
                if jj == 1:
                    return xt_t[:, 12 + k, 0:512]
                if jj == 5:
                    return xt_t[:, 12 + k, 512:1024]
                return [None, None, t2_t, t3_t, t1_t, None, t4_t][jj][:, k, :]

            def bias_ap(gate, m):
                c = BIAS_IDX[gate] * MT + m
                return bt_t[:, c:c + 1]

            i_gates = {}
            th_tiles = {}
            fc_tiles = {}
            hc_tiles = {}

            def straight_gemm(wk, nk, koff, m):
                """k-outer/n-inner accumulation, returns NT psum banks."""
                ps = [p_pool.tile([128, NFREE], F32, name=f"ps_{m}_{n}",
                                  tag="ps") for n in range(NT)]
                for k in range(nk):
                    for n in range(NT):
                        nc.tensor.matmul(
                            ps[n][:], wk(k),
                            xt_t[:, koff + k, n * NFREE:(n + 1) * NFREE],
                            start=(k == 0), stop=(k == nk - 1))
                return ps

            pf_sb_tiles = {}

            def pf_compute_pair(pi):
                """pf gemms for the m-pair (pi, pi+4), parked in SBUF f16
                until the pair's fl/fr run.

                Hoisted one iteration ahead: pf only needs xt k-tiles 0..7
                (the first 2MB), so it is the only real PE work available
                while the 6.3MB xt stream lands at the kernel front.  The
                k-loop interleaves both m's (4 matmuls per k-tile) so pf
                consumes the arriving xt ladder at the DMA rate instead of
                stalling on it.  DVE reads only one PSUM operand per op,
                hence the ACT copies.
                """
                wpf_t = wpf_tiles[pi]
                ms = (pi, pi + 4)
                ps = {m: [p_pool.tile([128, NFREE], F32,
                                      name=f"ps_pf_{m}_{n}", tag="ps")
                          for n in range(NT)] for m in ms}
                for k in range(8):
                    for mi, m in enumerate(ms):
                        for n in range(NT):
                            nc.tensor.matmul(
                                ps[m][n][:], wpf_t[:, mi * 8 + k, :],
                                xt_t[:, k, n * NFREE:(n + 1) * NFREE],
                                start=(k == 0), stop=(k == 7))
                for m in ms:
                    pf_sb = []
                    for n in range(NT):
                        t = g_pool.tile([128, NFREE], F16,
                                        name=f"pfsb_{m}_{n}", tag="pfsb",
                                        bufs=8)
                        nc.scalar.copy(t[:], ps[m][n][:])
                        pf_sb.append(t)
                    pf_sb_tiles[m] = pf_sb

            def flfr_compute(m):
                pf_sb = pf_sb_tiles[m]
                wfl_t = flfr_tiles[m]
                gates = {}
                for gname, off in (("fl", 0), ("fr", 16)):
                    ps = straight_gemm(
                        lambda k, _o=off: wfl_t[:, _o + k, :], 16, 8, m)
                    gt = g_pool.tile([128, BL], F16, name=f"g_{gname}_{m}",
                                     tag=f"g{gname}", bufs=2)
                    for n in range(NT):
                        nc.vector.tensor_add(ps[n][:], ps[n][:],
                                             pf_sb[n][:])
                        nc.scalar.activation(
                            gt[:, n * NFREE:(n + 1) * NFREE], ps[n][:],
                            SIG, bias=bias_ap(gname, m))
                    gates[gname] = gt
                # fc = fl*cl + fr*cr, built in place over the cl half of
                # the clcr tile (cl is dead after the first mul; fr's gate
                # tile is consumed in place by the second)
                clcr_t = clcr_tiles[m]
                nc.vector.tensor_mul(clcr_t[:, 0:BL], gates["fl"][:],
                                     clcr_t[:, 0:BL])
                nc.vector.tensor_mul(gates["fr"][:], gates["fr"][:],
                                     clcr_t[:, BL:2 * BL])
                nc.vector.tensor_add(clcr_t[:, 0:BL], clcr_t[:, 0:BL],
                                     gates["fr"][:])
                fc_tiles[m] = clcr_t

            def act_block(gate, bank, m, out_ap):
                fn = TANH if gate == "u" else SIG
                nc.scalar.activation(out_ap, bank[:], fn,
                                     bias=bias_ap(gate, m))

            def u_chain(bank, m, n):
                """ACT(u) -> c -> tanh for C-block (m, n)."""
                gu = g_pool.tile([128, NFREE], F16, name=f"gu_{m}_{n}",
                                 tag="gact", bufs=2)
                act_block("u", bank, m, gu[:])
                sf = slice(n * NFREE, (n + 1) * NFREE)
                nc.vector.tensor_mul(gu[:], gu[:], i_gates[m][:, sf])
                hc = hc_tiles[m]
                nc.vector.tensor_add(hc[:, sf], gu[:], fc_tiles[m][:, sf])
                th = th_tiles[m]
                nc.scalar.activation(th[:, sf], hc[:, sf], TANH)

            def o_block(bank, m, n):
                og = g_pool.tile([128, NFREE], F16, name=f"og_{m}_{n}",
                                 tag="gact", bufs=2)
                act_block("o", bank, m, og[:])
                sf_h = slice(BL + n * NFREE, BL + (n + 1) * NFREE)
                sf = slice(n * NFREE, (n + 1) * NFREE)
                nc.vector.tensor_mul(hc_tiles[m][:, sf_h], og[:],
                                     th_tiles[m][:, sf])

            def emit_block(gate, bank, ph, blk):
                """Dispatch a finished C-block: blk in {1:C11, 4:C22,
                5:C12, 6:C21} of the fold schedule's bank index."""
                m, n = {1: (ph, 0), 4: (ph + 4, 1),
                        5: (ph, 1), 6: (ph + 4, 0)}[blk]
                if gate == "i":
                    act_block("i", bank, m,
                              i_gates[m][:, n * NFREE:(n + 1) * NFREE])
                elif gate == "u":
                    u_chain(bank, m, n)
                else:
                    o_block(bank, m, n)

            def strassen_group(g, gate, ph):
                """7 products + folds.  bk = [M1, M2, M6, M7, M5, M3, M4].

                DVE reads at most one PSUM operand per op: M1 is ACT-staged
                to SBUF (s1); the intermediate U2/U3/U4 are computed
                PSUM+SBUF -> SBUF f16 (freeing their banks immediately);
                the four C-blocks land PSUM+SBUF -> PSUM for the ACT.
                """
                wst = ws_tiles[(g, ph)]
                bk = []
                st = {}

                def stage(i):
                    t = g_pool.tile([128, NFREE], F16,
                                    name=f"s{i}_{gate}_{ph}", tag="stg",
                                    bufs=3)
                    st[i] = t
                    return t

                for jj in range(NJJ):
                    ps = p_pool.tile([128, NFREE], F32,
                                     name=f"ps_{gate}_{ph}_{jj}", tag="ps")
                    for k in range(KJJ):
                        nc.tensor.matmul(ps[:], wst[:, jj * KJJ + k, :],
                                         moving(jj, k),
                                         start=(k == 0), stop=(k == KJJ - 1))
                    bk.append(ps)
                    if jj == 0:
                        nc.scalar.copy(stage(1)[:], bk[0][:])   # s1 = M1
                    elif jj == 1:
                        nc.vector.tensor_add(bk[1][:], bk[1][:], st[1][:])
                        emit_block(gate, bk[1], ph, 1)      # U1 = C11
                    elif jj == 2:
                        nc.vector.tensor_add(stage(2)[:], bk[2][:],
                                             st[1][:])      # U2 -> SBUF
                    elif jj == 3:
                        nc.vector.tensor_add(stage(3)[:], bk[3][:],
                                             st[2][:])      # U3 -> SBUF
                    elif jj == 4:
                        nc.vector.tensor_add(stage(4)[:], bk[4][:],
                                             st[2][:])      # U4 -> SBUF
                        nc.vector.tensor_add(bk[4][:], bk[4][:], st[3][:])
                        emit_block(gate, bk[4], ph, 4)      # U7 = C22
                    elif jj == 5:
                        nc.vector.tensor_add(bk[5][:], bk[5][:], st[4][:])
                        emit_block(gate, bk[5], ph, 5)      # U5 = C12
                    elif jj == 6:
                        nc.vector.tensor_sub(bk[6][:], st[3][:], bk[6][:])
                        emit_block(gate, bk[6], ph, 6)      # U6 = C21
                return bk

            wo3_t = None
            for ph in range(4):
                last = ph == 3
                for m in (ph, ph + 4):
                    i_gates[m] = g_pool.tile([128, BL], F16,
                                             name=f"g_i_{m}", tag="gi",
                                             bufs=2)
                    th_tiles[m] = e_pool.tile([128, BL], F16,
                                              name=f"th_{m}", tag="th",
                                              bufs=2)
                    hc_tiles[m] = e_pool.tile([128, 2 * BL], F16,
                                              name=f"hc_{m}", tag="hc",
                                              bufs=2)
                # pf for the NEXT iteration's m-pair runs at this iter's
                # front (kernel front: both of iter0+iter1's pairs)
                if ph == 0:
                    pf_compute_pair(0)
                    pf_compute_pair(1)
                elif not last:
                    pf_compute_pair(ph + 1)
                flfr_compute(ph)
                # flfr ring slot freed by the gemms just emitted: a trigger
                # emitted here never waits on a future iteration (deadlock)
                if ph + 1 < 4:
                    load_flfr(ph + 1)
                if ph < 2:
                    load_wpf_pair(ph + 2)

                strassen_ws_after = {
                    0: [(2, 0), (0, 1), (1, 1)],
                    1: [(2, 1), (0, 2), (1, 2)],
                    2: [(2, 2), (0, 3), (1, 3)],
                    3: [None, None, None],
                }[ph]
                flfr_compute(ph + 4)
                if ph == 0:
                    # 3.3us each on the in-order DVE queue: emitted after
                    # ALL iter0 fl/fr folds (which gate PSUM bank drains /
                    # the i-group's bank supply); the PE reads T3 at ~53us
                    # and T4 at ~59us, both comfortably met
                    nc.vector.tensor_sub(t3_t[:], b22, b12)
                    nc.vector.tensor_sub(t4_t[:], t2_t[:], b21)
                if ph + 1 < 4:
                    load_flfr(ph + 5)
                strassen_group(0, "i", ph)
                if strassen_ws_after[0] is not None:
                    load_ws(*strassen_ws_after[0])
                elif last:
                    wo3_t = w_pool.tile([128, NJJ * KJJ, 128], F16,
                                        name="wo3", tag="ws", bufs=2)
                    nc.sync.dma_start(
                        wo3_t[:, 0:48, :],
                        wo3_d.rearrange("p (k c) -> p k c", k=48))
                strassen_group(1, "u", ph)
                if strassen_ws_after[1] is not None:
                    load_ws(*strassen_ws_after[1])
                # clcr(ph) is freed by the c-chain just emitted (fc lives in
                # the clcr tile); next-iter triggers must come after the ws
                # trigger above to keep Sync dep times monotone
                if ph + 1 < 4:
                    load_clcr(ph + 1)
                    load_clcr(ph + 5)
                if not last:
                    strassen_group(2, "o", ph)
                    if strassen_ws_after[2] is not None:
                        load_ws(*strassen_ws_after[2])
                    nc.sync.dma_start(
                        hct_d[ph * 128:(ph + 1) * 128, :],
                        hc_tiles[ph][:])
                    nc.sync.dma_start(
                        hct_d[(ph + 4) * 128:(ph + 5) * 128, :],
                        hc_tiles[ph + 4][:])
                else:
                    # straight o for m=3 and m=7: short, fine-grained tail
                    for mi, m in enumerate((3, 7)):
                        sp = slice(m * 128, (m + 1) * 128)
                        for n in range(NT):
                            ps = p_pool.tile([128, NFREE], F32,
                                             name=f"ps_o_{m}_{n}",
                                             tag="ps")
                            for k in range(KT):
                                nc.tensor.matmul(
                                    ps[:],
                                    wo3_t[:, mi * 24 + k, :],
                                    xt_t[:, k, n * NFREE:(n + 1) * NFREE],
                                    start=(k == 0), stop=(k == KT - 1))
                            o_block(ps, m, n)
                            if m == 7 and n == 0:
                                nc.sync.dma_start(
                                    hct_d[sp, 0:BL + NFREE],
                                    hc_tiles[m][:, 0:BL + NFREE])
                        if m == 3:
                            nc.sync.dma_start(hct_d[sp, :],
                                              hc_tiles[m][:])
                        else:
                            nc.sync.dma_start(
                                hct_d[sp, BL + NFREE:2 * BL],
                                hc_tiles[m][:, BL + NFREE:2 * BL])

    nc.compile()
    return nc


def _get_program():
    if "nc" not in _CACHE:
        _CACHE["nc"] = _build_program()
    return _CACHE["nc"]


def _tile_weight(V, nk):
    """[nk*128, H] -> [MT, 128, nk*128] with [m][kp, k*128+mc] = V[k*128+kp, m*128+mc]."""
    return np.ascontiguousarray(
        V.reshape(nk, 128, MT, 128)
         .transpose(2, 1, 0, 3)
         .reshape(MT, 128, nk * 128))


def _strassen_pack(V):
    """V [3072, 1024] (k, h) -> [4, 128, 84*128] A-side Winograd tiles.

    Product jj, k-tile k, row-group ph: tile[ph][:, (jj*12+k)*128:+128] =
    G_jj.T[k*128:+128, ph*128:+128], G order A11,A12,S2,S3,S1,S4,A22.
    """
    A = V.T
    A11, A12 = A[:512, :1536], A[:512, 1536:]
    A21, A22 = A[512:, :1536], A[512:, 1536:]
    S1 = A21 + A22
    S2 = S1 - A11
    S3 = A11 - A21
    S4 = A12 - S2
    G = np.stack([A11, A12, S2, S3, S1, S4, A22])   # [7, 512, 1536]
    G = G.reshape(7, 4, 128, 12, 128)               # [jj, ph, hc, k, kp]
    return np.ascontiguousarray(
        G.transpose(1, 4, 0, 3, 2).reshape(4, 128, NJJ * KJJ * 128))


def kernel(hl, cl, hr, cr, p,
           Wd, Wdl, Wdr, bd,
           Wf, Wfll, Wflr, Wfrl, Wfrr, bfl, bfr,
           Wo, Wol, Wor, bo,
           Wi, Wil, Wir, bi):
    global LAST_RESULTS
    f32 = np.float32
    f16 = np.float16
    hl, cl, hr, cr, p = (np.asarray(a, dtype=f32) for a in (hl, cl, hr, cr, p))
    ws = {k: np.asarray(v, dtype=f32) for k, v in dict(
        Wd=Wd, Wdl=Wdl, Wdr=Wdr, Wf=Wf, Wfll=Wfll, Wflr=Wflr, Wfrl=Wfrl,
        Wfrr=Wfrr, Wo=Wo, Wol=Wol, Wor=Wor, Wi=Wi, Wil=Wil, Wir=Wir).items()}

    # Wf{gate l/r}{child l/r}: f_left mixes hl via Wfll and hr via Wflr;
    # f_right mixes hl via Wfrl and hr via Wfrr.
    Vi = np.concatenate([ws["Wd"].T, ws["Wdl"].T, ws["Wdr"].T], 0)
    Vu = np.concatenate([ws["Wi"].T, ws["Wil"].T, ws["Wir"].T], 0)
    Vo = np.concatenate([ws["Wo"].T, ws["Wol"].T, ws["Wor"].T], 0)
    wstr = np.concatenate([_strassen_pack(V) for V in (Vi, Vu, Vo)],
                          axis=0).astype(f16)          # [12, 128, 84*128]
    wt_fl = _tile_weight(np.concatenate([ws["Wfll"].T, ws["Wflr"].T], 0), 16)
    wt_fr = _tile_weight(np.concatenate([ws["Wfrl"].T, ws["Wfrr"].T], 0), 16)
    flfr = np.concatenate([wt_fl, wt_fr], axis=2).astype(f16)
    wpf_m = _tile_weight(np.ascontiguousarray(ws["Wf"].T), 8)
    wpf = np.ascontiguousarray(np.stack(
        [np.concatenate([wpf_m[pi], wpf_m[pi + 4]], axis=1)
         for pi in range(4)])).astype(f16)       # [4, 128, 16*128]
    wt_o = _tile_weight(Vo, KT)
    wo3 = np.ascontiguousarray(
        np.concatenate([wt_o[3], wt_o[7]], axis=1)).astype(f16)

    Bt = np.empty((128, 5 * MT), dtype=f32)
    for name, b_ in (("i", bd), ("fl", bfl), ("fr", bfr), ("u", bi), ("o", bo)):
        gi = BIAS_IDX[name]
        Bt[:, gi * MT:(gi + 1) * MT] = np.asarray(b_, dtype=f32).reshape(MT, 128).T

    X = np.concatenate([p, hl, hr], axis=1)    # [B, 3D]

    in_maps = []
    for r in range(NCORES):
        rows = slice(r * BL, (r + 1) * BL)
        clcr = np.concatenate([cl[rows].T, cr[rows].T], axis=1)
        im = {
            "xt": np.ascontiguousarray(X[rows].T.astype(f16)),
            "w_str": wstr,
            "w_flfr": flfr,
            "w_pf": wpf,
            "w_o3": wo3,
            "clcr": np.ascontiguousarray(clcr.astype(f16)),
            "bt": Bt,
        }
        in_maps.append(im)

    nc = _get_program()
    res = bass_utils.run_bass_kernel_spmd(nc, in_maps,
                                          core_ids=list(range(NCORES)))
    LAST_RESULTS = res

    h = np.empty((B, H), dtype=f32)
    c = np.empty((B, H), dtype=f32)
    for r in range(NCORES):
        rows = slice(r * BL, (r + 1) * BL)
        hct = res.results[r]["hct"]
        c[rows] = hct[:, 0:BL].T.astype(f32)
        h[rows] = hct[:, BL:2 * BL].T.astype(f32)
    return (h, c)
